# revision 2
# baseline (speedup 1.0000x reference)
"""Trainium2 Bass kernel for quantized Conv1D forward:
    y = x @ (w_q * scale) + bias
  x:     [4, 2048, 4096] f32
  w_q:   [4096, 16384] int32 (values in [-127, 127])
  scale: [16384] f32
  bias:  [16384] f32
  y:     [4, 2048, 16384] f32

Sharding: column-parallel over out_features across 8 cores (N=2048 each);
x replicated. Each core computes y_shard = x @ (w_q_shard * scale_shard)
+ bias_shard independently (no collectives); host concatenates shards.

Device strategy (single-pass fp16):
  - scale is folded into the weights on the host: wh = fp16(w_q * scale).
    x is cast to fp16. One fp16 matmul pass accumulating in fp32 PSUM gives
    ~4e-4 rel err vs the f32 reference -- far inside the 2e-2 gate -- at
    half the PE work of the previous hi+lo x-split scheme.
  - The fp16 weight shard [4096, 2048] stays fully resident in SBUF
    (128 KB/partition); x tiles stream through; PE runs back-to-back
    matmuls (stationary = x^T tile, moving = w rows).
  - Per tb tile: 256-wide matmuls accumulate K=4096 into 4 PSUM banks x 2
    sub-regions; DVE adds bias (PSUM -> SBUF); DMA writes y rows.
"""

import numpy as np

import concourse.bass as bass
import concourse.mybir as mybir
import concourse.tile as tile
from concourse import bacc
from concourse.bass import ts
from concourse.bass_utils import run_bass_kernel_spmd

P = 128
N_CORES = 8


def build_nc(T, K, N, n_free=512, reps=1,
             x_bufs=2, o_bufs=2, p_bufs=None, x_dma_split=1, swap_loop=False,
             mm_dt="fp16", w_split=False, y_dma_split=1, n_sub=1, sub_order="A"):
    """Build the per-core Bass program.

    DRAM I/O (per core):
      xh:    [TB, P, KB, Tt] fp16  packed x^T tiles
      wh:    [P, KB, N]      fp16  scale-folded weight shard, k on partitions
      bias:  [N] f32
      y:     [T, N] f32 out
    """
    KB = K // P
    TB = T // P
    Tt = P
    NB = N // n_free
    mdt = {"fp16": mybir.dt.float16, "bf16": mybir.dt.bfloat16,
           "fp32r": mybir.dt.float32r}[mm_dt]

    nc = bacc.Bacc("TRN2", target_bir_lowering=False, debug=False)

    xh = nc.dram_tensor("xh", [TB, P, KB, Tt], mdt, kind="ExternalInput")
    wh = nc.dram_tensor("wh", [P, KB, N], mdt, kind="ExternalInput")
    bias_h = nc.dram_tensor("bias", [N], mybir.dt.float32, kind="ExternalInput")
    y_h = nc.dram_tensor("y", [T, N], mybir.dt.float32, kind="ExternalOutput")

    xh_ap = xh.ap()
    wh_ap = wh.ap()
    y_ap = y_h.ap().rearrange("(tb p) n -> tb p n", p=P)

    def bcast_ap(ap):
        # [N] dram vector -> [P, N] with step-0 partition dim for DMA broadcast
        return bass.AP(tensor=ap.tensor, offset=ap.offset, ap=[[0, P], *ap.ap])

    with tile.TileContext(nc) as tc:
        if p_bufs is None:
            p_bufs = 2 * NB
        with (
            tc.tile_pool(name="wpool", bufs=1) as wpool,
            tc.tile_pool(name="cpool", bufs=1) as cpool,
            tc.tile_pool(name="xpool", bufs=x_bufs) as xpool,
            tc.tile_pool(name="opool", bufs=o_bufs) as opool,
            tc.tile_pool(name="ppool", bufs=p_bufs, space="PSUM") as ppool,
        ):
            # resident weights: [P, KB, N]; per-kb tiles give per-slice deps
            if w_split:
                w_tiles = []
                for kb in range(KB):
                    wt = wpool.tile([P, N], mdt, name=f"w{kb}")
                    nc.sync.dma_start(out=wt[:], in_=wh_ap[:, kb])
                    w_tiles.append(wt)
                w_rhs = lambda kb, c0, w: w_tiles[kb][:, c0:c0 + w]
            else:
                w_res = wpool.tile([P, KB, N], mdt, name="w_res")
                for kb in range(KB):
                    nc.sync.dma_start(out=w_res[:, kb], in_=wh_ap[:, kb])
                w_rhs = lambda kb, c0, w: w_res[:, kb, c0:c0 + w]

            bias_b = cpool.tile([P, N], mybir.dt.float32, name="bias_b")
            nc.sync.dma_start(out=bias_b[:], in_=bcast_ap(bias_h.ap()))

            for tb in [t for _ in range(reps) for t in range(TB)]:
                xt = xpool.tile([P, KB, Tt], mdt, tag="xt")
                if x_dma_split == 1:
                    nc.sync.dma_start(out=xt[:], in_=xh_ap[tb])
                else:
                    assert KB % x_dma_split == 0
                    c = KB // x_dma_split
                    for d in range(x_dma_split):
                        nc.sync.dma_start(
                            out=xt[:, ts(d, c)],
                            in_=xh_ap[tb, :, ts(d, c)],
                        )

                psums = [
                    ppool.tile([P, n_free], mybir.dt.float32, tag="acc", name=f"ps{nb}")
                    for nb in range(NB)
                ]
                # n_sub>1: split each PSUM bank into n_sub independent
                # accumulation regions (has_written is per-element), raising
                # stationary reuse from NB to NB*n_sub MMs per weight load
                w_free = n_free // n_sub  # moving free per MM
                if n_sub > 1:
                    if sub_order == "A":  # slice-major: cycle banks, then halves
                        sl_iter = [(b, h) for h in range(n_sub) for b in range(NB)]
                    else:  # "B" bank-major: both halves of a bank back-to-back
                        sl_iter = [(b, h) for b in range(NB) for h in range(n_sub)]
                    mm_iter = [
                        (kb, b, h)
                        for kb in range(KB)
                        for (b, h) in sl_iter
                    ]
                elif swap_loop:
                    mm_iter = [
                        (kb, nb, 0)
                        for nb in range(NB)
                        for kb in range(KB)
                    ]
                else:
                    mm_iter = [
                        (kb, nb, 0)
                        for kb in range(KB)
                        for nb in range(NB)
                    ]
                for kb, b, h in mm_iter:
                    nc.tensor.matmul(
                        psums[b][:, ts(h, w_free)],
                        lhsT=xt[:, kb, :],
                        rhs=w_rhs(kb, b * n_free + h * w_free, w_free),
                        start=(kb == 0 and h == 0),
                        stop=(kb == KB - 1),
                        skip_group_check=(n_sub > 1),
                    )

                out_sb = opool.tile([P, N], mybir.dt.float32, tag="out")
                for nb in range(NB):
                    nc.vector.tensor_add(
                        out=out_sb[:, ts(nb, n_free)],
                        in0=psums[nb][:],
                        in1=bias_b[:, ts(nb, n_free)],
                    )
                if y_dma_split == 1:
                    nc.sync.dma_start(out=y_ap[tb], in_=out_sb[:])
                else:
                    c = N // y_dma_split
                    for d in range(y_dma_split):
                        nc.sync.dma_start(
                            out=y_ap[tb, :, ts(d, c)], in_=out_sb[:, ts(d, c)]
                        )

    nc.compile()
    return nc


def pack_x(x2d, T, K, np_dt=np.float16):
    """[T, K] f32 -> [TB, P, KB, Tt] tiles of x^T in fp16."""
    TB, KB = T // P, K // P
    # [T, K] -> [TB, Tt, KB, Pk] -> [TB, Pk, KB, Tt]
    return np.ascontiguousarray(
        x2d.astype(np_dt).reshape(TB, P, KB, P).transpose(0, 3, 2, 1)
    )


def pack_w(w_shard, scale_shard, K, N, np_dt=np.float16):
    """[K, N] int + [N] scale -> fp16 [P, KB, N] with scale folded in."""
    KB = K // P
    w = w_shard.astype(np.float32) * scale_shard[None, :].astype(np.float32)
    return np.ascontiguousarray(
        w.astype(np_dt).reshape(KB, P, N).transpose(1, 0, 2)
    )


def make_in_maps(x, w_q, scale, bias):
    """Full inputs -> per-core in_maps (column-parallel over out_features)."""
    x = np.asarray(x)
    w_q = np.asarray(w_q)
    scale = np.asarray(scale, dtype=np.float32)
    bias = np.asarray(bias, dtype=np.float32)
    B, Sq, K = x.shape
    K2, D_OUT = w_q.shape
    assert K2 == K
    T = B * Sq
    N = D_OUT // N_CORES

    xh = pack_x(np.ascontiguousarray(x.reshape(T, K)), T, K)
    in_maps = []
    for c in range(N_CORES):
        sl = slice(c * N, (c + 1) * N)
        in_maps.append(
            {
                "xh": xh,
                "wh": pack_w(w_q[:, sl], scale[sl], K, N),
                "bias": np.ascontiguousarray(bias[sl], dtype=np.float32),
            }
        )
    return in_maps, (B, Sq, T, K, N, D_OUT)


_NC_CACHE = {}

# tuned on hardware: x DMA in 4 chunks + two independent 256-wide
# accumulation regions per PSUM bank (stationary reused 8 MMs per load)
TUNED = dict(x_dma_split=4, n_sub=2, sub_order="B")


def _get_nc(T, K, N):
    key = (T, K, N)
    if key not in _NC_CACHE:
        _NC_CACHE[key] = build_nc(T, K, N, **TUNED)
    return _NC_CACHE[key]


def kernel(x, w_q, scale, bias):
    in_maps, (B, Sq, T, K, N, D_OUT) = make_in_maps(x, w_q, scale, bias)
    nc = _get_nc(T, K, N)
    res = run_bass_kernel_spmd(nc, in_maps, core_ids=list(range(N_CORES)))
    y = np.concatenate([r["y"] for r in res.results], axis=1)
    return y.reshape(B, Sq, D_OUT)
